# revision 18
# baseline (speedup 1.0000x reference)
"""MMoE-style CustomizedGateControl kernel for 8x TRN2 NeuronCores.

Data-parallel over the batch dim (16384 -> 8 x 2048). Per core:
  - expert GEMMs in groups of 4 b-tiles, third-major within a group so a
    whole group's 12 expert outputs complete early and its gated combine
    overlaps the next group's GEMMs.
  - bias lands in PSUM before the matmul group accumulates onto it:
    group 0 via a 1-partition ones x bias-row matmul, later groups via
    DMA preload (SBUF->PSUM) issued far ahead on the gpsimd queue. The
    drain is then a single fused relu+downcast activation per psum tile.
  - gates as gw-stationary GEMMs -> [16, 512] psum per 512-row batch
    chunk, drained to f16 and flipped by a hardware DMA transpose into
    [128b, (i,16)] so gate values become per-partition scalars.
  - gated combine per b-tile: most b-tiles as DVE FMA chains
    (scalar_tensor_tensor with per-partition gate scalar), the last
    groups as PE matmuls with diag(gate) stationary. Both paths produce
    info [b, t*256] f16 in SBUF; one DMA transpose per (i,t) flips the
    slab into infoT [e, b] for the towers.
  - tower MLP per (task, 512-col batch chunk): 2 PE GEMMs + ACT relu
    (per-partition bias) + 1 PE GEMM, output DMA'd straight from PSUM.
All parameters replicated; no collectives.
"""

import sys

if "/opt/trn_rl_repo" not in sys.path:
    sys.path.insert(0, "/opt/trn_rl_repo")

import numpy as np

import concourse.bacc as bacc
import concourse.mybir as mybir
import concourse.tile as tile
from concourse.bass_utils import run_bass_kernel_spmd

# problem dims
B, D, E, H = 16384, 512, 256, 128
S, K, T = 4, 4, 2
NCORES = 8
BC = B // NCORES          # 2048 batch rows per core
P = 128                   # partitions
NB = BC // P              # 16 b-tiles per core
NE = S + T * K            # 12 experts
G = S + K                 # 8 gate inputs per task
WCOLS = NE * E            # 3072 expert output columns
WALL = WCOLS + T * G      # 3088 = experts + gate columns
KC = D // P               # 4 contraction chunks
NTH = WCOLS // 512        # 6 psum thirds per b-tile

f32 = mybir.dt.float32
f16 = mybir.dt.float16
RELU = mybir.ActivationFunctionType.Relu
COPY = mybir.ActivationFunctionType.Copy
MULT = mybir.AluOpType.mult
ADD = mybir.AluOpType.add

# b-tiles >= PE_CUT are combined on the tensor engine (diag-stationary
# matmuls); the rest run as DVE FMA chains overlapped with later groups.
PE_CUT = 12


def _expert_col(t: int, g: int) -> int:
    """Column offset of expert g-of-task-t in the fused expert output."""
    if g < S:
        return g * E                      # shared expert g
    return (S + t * K + (g - S)) * E      # task expert (t, g-S)


def _build():
    nc = bacc.Bacc("TRN2", target_bir_lowering=False, debug=False)

    xt_d = nc.dram_tensor("xt", [D, BC], f16, kind="ExternalInput").ap()
    wall_d = nc.dram_tensor("wall", [D, WALL], f16, kind="ExternalInput").ap()
    brow_d = nc.dram_tensor("brow", [1, WCOLS], f16, kind="ExternalInput").ap()
    tw1_d = nc.dram_tensor("tw1", [T, E, H], f16, kind="ExternalInput").ap()
    tb1_d = nc.dram_tensor("tb1", [H, T], f32, kind="ExternalInput").ap()
    tw2_d = nc.dram_tensor("tw2", [H, T], f16, kind="ExternalInput").ap()
    ident_d = nc.dram_tensor("ident", [P, P], f16, kind="ExternalInput").ap()
    out_d = nc.dram_tensor("out", [T, BC], f32, kind="ExternalOutput").ap()

    with tile.TileContext(nc) as tc:
        with (
            tc.tile_pool(name="const", bufs=1) as const,
            tc.tile_pool(name="acc", bufs=6) as acc_pool,
            tc.tile_pool(name="isb", bufs=2) as isb_pool,
            tc.tile_pool(name="hsb", bufs=2) as hsb_pool,
        ):
            xt_t = [const.tile([P, BC], f16, tag=f"xt{k}", name=f"xt{k}") for k in range(KC)]
            wall_t = [const.tile([P, WALL], f16, tag=f"wall{k}", name=f"wall{k}") for k in range(KC)]
            brow = const.tile([1, WCOLS], f16, tag="brow", name="brow")
            ones = const.tile([1, P], f16, tag="ones", name="ones")
            ident = const.tile([P, P], f16, tag="ident", name="ident")
            exp_sb = [
                const.tile([P, WCOLS], f16, tag=f"expsb{i}", name=f"expsb{i}")
                for i in range(NB)
            ]
            gtsb = const.tile([T * G, BC], f16, tag="gtsb", name="gtsb")
            gsb = const.tile([P, NB * T * G], f16, tag="gsb", name="gsb")
            gsb32 = const.tile([P, NB * T * G], f32, tag="gsb32", name="gsb32")
            infoT = const.tile([P, T * 2 * BC], f16, tag="infoT", name="infoT")
            diag_t = {
                i: const.tile([P, T * G * P], f16, tag=f"diag{i}", name=f"diag{i}")
                for i in range(PE_CUT, NB)
            }
            tw1_t = {}
            for t in range(T):
                for kc in range(2):
                    t_ = const.tile([P, H], f16, tag=f"tw1_{t}_{kc}", name=f"tw1_{t}_{kc}")
                    tw1_t[(t, kc)] = t_
            tb1 = const.tile([H, T], f32, tag="tb1", name="tb1")
            tw2 = const.tile([H, T], f16, tag="tw2", name="tw2")
            out_sb = const.tile([1, T * BC], f32, tag="out_sb", name="out_sb")

            nc.vector.memset(ones[:], 1.0)

            # ---- input DMAs, first-use order ----
            # gpsimd: xt; sync: wall (third-major); scalar: gate cols,
            # bias row + small consts.
            for k in range(KC):
                rs = slice(k * P, (k + 1) * P)
                nc.gpsimd.dma_start(xt_t[k][:, 0:512], xt_d[rs, 0:512])
                nc.sync.dma_start(wall_t[k][:, 0:512], wall_d[rs, 0:512])
            nc.scalar.dma_start(brow[:], brow_d[:])
            for k in range(KC):
                rs = slice(k * P, (k + 1) * P)
                nc.scalar.dma_start(wall_t[k][:, WCOLS:WALL], wall_d[rs, WCOLS:WALL])
            for k in range(KC):
                rs = slice(k * P, (k + 1) * P)
                nc.gpsimd.dma_start(xt_t[k][:, 512:1024], xt_d[rs, 512:1024])
                nc.gpsimd.dma_start(xt_t[k][:, 1024:BC], xt_d[rs, 1024:BC])
            for third in range(1, NTH):
                cs = slice(third * 512, (third + 1) * 512)
                for k in range(KC):
                    rs = slice(k * P, (k + 1) * P)
                    nc.sync.dma_start(wall_t[k][:, cs], wall_d[rs, cs])
            nc.scalar.dma_start(ident[:], ident_d[:])
            for t in range(T):
                for kc in range(2):
                    nc.scalar.dma_start(
                        tw1_t[(t, kc)][:], tw1_d[t, kc * P : (kc + 1) * P, :]
                    )
            nc.scalar.dma_start(tb1[:], tb1_d[:])
            nc.scalar.dma_start(tw2[:], tw2_d[:])

            with (
                tc.tile_pool(name="expps", bufs=3, space="PSUM") as expps_pool,
                tc.tile_pool(name="gateps", bufs=1, space="PSUM") as gateps_pool,
                tc.tile_pool(name="infops", bufs=2, space="PSUM") as infops_pool,
                tc.tile_pool(name="hps", bufs=1, space="PSUM") as hps_pool,
                tc.tile_pool(name="ops", bufs=1, space="PSUM") as ops_pool,
            ):
                infoT_v = infoT[:].rearrange("p (q b) -> p q b", b=BC)
                gsb_v = gsb[:].rearrange("p (i j) -> p i j", j=T * G)
                gsb32_v = gsb32[:].rearrange("p (i j) -> p i j", j=T * G)

                def emit_gates(bc):
                    bs = slice(bc * 512, (bc + 1) * 512)
                    gp = gateps_pool.tile([T * G, 512], f32, tag="gateps", name="gateps")
                    for k in range(KC):
                        nc.tensor.matmul(
                            gp[:],
                            wall_t[k][:, WCOLS:WALL],
                            xt_t[k][:, bs],
                            start=(k == 0),
                            stop=(k == KC - 1),
                        )
                    nc.vector.tensor_copy(gtsb[:, bs], gp[:])
                    # hw DMA transpose: [16, 512] -> [128, (4 b-tiles, 16)]
                    nc.sync.dma_start_transpose(
                        gsb_v[:, bc * 4 : (bc + 1) * 4, :], gtsb[:, bs]
                    )
                    gcs = slice(bc * 4 * T * G, (bc + 1) * 4 * T * G)
                    nc.vector.tensor_copy(gsb32[:, gcs], gsb[:, gcs])

                def emit_diags():
                    for i in sorted(diag_t):
                        dg = diag_t[i]
                        nc.vector.tensor_mul(
                            dg[:].rearrange("p (j c) -> p j c", c=P),
                            ident[:, None, :].broadcast_to([P, T * G, P]),
                            gsb_v[:, i, :, None].broadcast_to([P, T * G, P]),
                        )

                def emit_expert_tile(i, third):
                    bs = slice(i * P, (i + 1) * P)
                    cs = slice(third * 512, (third + 1) * 512)
                    pe = expps_pool.tile([P, 512], f32, tag="expps", name="expps")
                    nc.tensor.matmul(
                        pe[:], ones[0:1, :], brow[0:1, cs],
                        start=True, stop=False, skip_group_check=True,
                    )
                    for k in range(KC):
                        nc.tensor.matmul(
                            pe[:],
                            xt_t[k][:, bs],
                            wall_t[k][:, cs],
                            start=False,
                            stop=(k == KC - 1),
                            skip_group_check=True,
                        )
                    nc.scalar.activation(exp_sb[i][:, cs], pe[:], RELU)

                def emit_combine(i):
                    if i >= PE_CUT:
                        ip = infops_pool.tile([P, T * E], f32, tag="infops", name="infops")
                        dg = diag_t[i]
                        for t in range(T):
                            for g in range(G):
                                c = _expert_col(t, g)
                                nc.tensor.matmul(
                                    ip[:, t * E : (t + 1) * E],
                                    dg[:, (t * G + g) * P : (t * G + g + 1) * P],
                                    exp_sb[i][:, c : c + E],
                                    start=(g == 0),
                                    stop=(g == G - 1),
                                    skip_group_check=True,
                                )
                        src = isb_pool.tile([P, T * E], f16, tag="isb", name="isb")
                        nc.scalar.activation(src[:], ip[:], COPY)
                        srcs = [src[:, t * E : (t + 1) * E] for t in range(T)]
                    else:
                        srcs = []
                        for t in range(T):
                            acc = acc_pool.tile([P, E], f16, tag="acc", name="acc")
                            c0 = _expert_col(t, 0)
                            nc.vector.tensor_scalar_mul(
                                acc[:],
                                exp_sb[i][:, c0 : c0 + E],
                                gsb32_v[:, i, t * G : t * G + 1],
                            )
                            for g in range(1, G):
                                c = _expert_col(t, g)
                                nc.vector.scalar_tensor_tensor(
                                    acc[:],
                                    exp_sb[i][:, c : c + E],
                                    gsb32_v[:, i, t * G + g : t * G + g + 1],
                                    acc[:],
                                    op0=MULT,
                                    op1=ADD,
                                )
                            srcs.append(acc[:])
                    for t in range(T):
                        # [128b, 256e] -> infoT rows (t,ec), cols = b-tile i
                        nc.sync.dma_start_transpose(
                            infoT_v[:, t * 2 : t * 2 + 2, i * P : (i + 1) * P],
                            srcs[t],
                        )

                def emit_tower(t, bc):
                    bs = slice(bc * 512, (bc + 1) * 512)
                    hp = hps_pool.tile([P, 512], f32, tag="hps", name="hps")
                    for kc in range(2):
                        nc.tensor.matmul(
                            hp[:],
                            tw1_t[(t, kc)][:],
                            infoT_v[:, t * 2 + kc, bs],
                            start=(kc == 0),
                            stop=(kc == 1),
                        )
                    hs = hsb_pool.tile([P, 512], f16, tag="hsb", name="hsb")
                    nc.scalar.activation(hs[:], hp[:], RELU, bias=tb1[:, t : t + 1])
                    op = ops_pool.tile([1, 512], f32, tag="ops", name="ops")
                    nc.tensor.matmul(op[:], tw2[:, t : t + 1], hs[:], start=True, stop=True)
                    r = slice(t * BC + bc * 512, t * BC + (bc + 1) * 512)
                    nc.scalar.activation(out_sb[0:1, r], op[0:1, :], COPY)
                    nc.sync.dma_start(out_d[t : t + 1, bs], out_sb[0:1, r])

                # groups of 4 b-tiles, third-major inside a group; gates,
                # diags and towers threaded in where their inputs are ready
                # and their consumers are not yet scheduled.
                emit_gates(0)
                for grp in range(4):
                    i0 = grp * 4
                    for third in range(NTH):
                        for i in range(i0, i0 + 4):
                            emit_expert_tile(i, third)
                        if grp == 0 and third == 1:
                            emit_gates(1)
                        if grp == 0 and third == 4:
                            emit_gates(2)
                        if grp == 1 and third == 0:
                            emit_gates(3)
                            emit_diags()
                        if grp == 2 and third == 4:
                            emit_tower(0, 0)
                            emit_tower(1, 0)
                        if grp == 3 and third == 4:
                            emit_tower(0, 1)
                            emit_tower(1, 1)
                    for i in range(i0, i0 + 4):
                        emit_combine(i)
                emit_tower(0, 2)
                emit_tower(1, 2)
                emit_tower(0, 3)
                emit_tower(1, 3)

    nc.compile()
    return nc


_NC = None


def _get_nc():
    global _NC
    if _NC is None:
        _NC = _build()
    return _NC


def _prep_shared(shared_W, shared_b, task_W, task_b, gate_W, tower_W1, tower_b1, tower_W2):
    cols = [np.asarray(shared_W[s]) for s in range(S)]
    cols += [np.asarray(task_W[t, k]) for t in range(T) for k in range(K)]
    gwi = np.empty((D, T * G), np.float32)
    for t in range(T):
        gwi[:, t * G : (t + 1) * G] = np.asarray(gate_W[t])  # col t*G+g = gate (t, g)
    cols += [gwi]
    wall = np.ascontiguousarray(np.concatenate(cols, axis=1), dtype=np.float16)
    bias_all = np.concatenate(
        [np.asarray(shared_b).reshape(-1), np.asarray(task_b).reshape(-1)]
    ).astype(np.float32)
    brow = np.ascontiguousarray(bias_all[None, :], dtype=np.float16)
    tw1 = np.ascontiguousarray(tower_W1, dtype=np.float16)
    tb1 = np.ascontiguousarray(np.asarray(tower_b1).T, dtype=np.float32)   # [H, T]
    tw2 = np.ascontiguousarray(np.asarray(tower_W2)[:, :, 0].T, dtype=np.float16)  # [H, T]
    ident = np.eye(P, dtype=np.float16)
    return wall, brow, tw1, tb1, tw2, ident


def kernel(
    x,
    shared_W,
    shared_b,
    task_W,
    task_b,
    gate_W,
    tower_W1,
    tower_b1,
    tower_W2,
    tower_b2,
    _trace=False,
    _tmpdir=None,
):
    nc = _get_nc()
    x = np.asarray(x, dtype=np.float32)
    wall, brow, tw1, tb1, tw2, ident = _prep_shared(
        shared_W, shared_b, task_W, task_b, gate_W, tower_W1, tower_b1, tower_W2
    )
    in_maps = []
    for c in range(NCORES):
        xt = np.ascontiguousarray(x[c * BC : (c + 1) * BC, :].T.astype(np.float16))
        in_maps.append(
            {
                "xt": xt,
                "wall": wall,
                "brow": brow,
                "tw1": tw1,
                "tb1": tb1,
                "tw2": tw2,
                "ident": ident,
            }
        )
    kw = {}
    if _trace:
        kw = {"trace": True, "tmpdir": _tmpdir}
    res = run_bass_kernel_spmd(nc, in_maps, core_ids=list(range(NCORES)), **kw)
    out = np.concatenate([res.results[c]["out"] for c in range(NCORES)], axis=1)
    out = out + np.asarray(tower_b2, dtype=np.float32)[:, 0][:, None]
    result = out[:, :, None].astype(np.float32)  # [T, B, 1]
    if _trace:
        return result, res
    return result
